# revision 23
# baseline (speedup 1.0000x reference)
"""Trainium2 Bass kernel for NeuronLlama4VisionMLP (fused residual-add +
RMSNorm + up-proj + GELU + down-proj).

Distribution: data-parallel over the 16384 tokens -> 2048 tokens per core,
full weights replicated per core, no collectives.

Host side (cheap elementwise / repack prep):
  - h = x + residual  (this is also the module's second output)
  - per-token rsqrt(mean(h^2)+eps) scale and ln_w are folded into the
    device inputs: normed = h * s, W_up' = ln_w[:,None] * W_up
  - normed is shipped transposed ([H, T] per core, fp16) so the device is
    a pure matmul pipeline; the device returns out^T and b_down is added
    on host.

Device side per core (T=2048 tokens, H=1408, I=5632), per TB-token block:
    up:   psum[i_tile, tok] = sum_k W_up'[k, i_tile].T @ normed_T[k, tok]
    gelu: act[i_tile] = Gelu(psum + b_up[i_tile])       (ACT engine)
    down: psum[m_tile, tok] = sum_i W_down[i, m_tile].T @ act[i]
    out^T[m_tile, tok] -> HBM

All matmul operands are fp16 (10-bit mantissa keeps fro rel-err ~4e-4):
fp16 weights enable fast weight load so LDWEIGHTS fully hides under the
512-column matmul (216 ns issue cadence vs 227 ns with f32r weights),
and halve HBM traffic vs f32.
"""
import sys

sys.path.insert(0, "/opt/trn_rl_repo")

import numpy as np
import ml_dtypes
import concourse.bass as bass
from concourse import bacc
import concourse.mybir as mybir
from concourse.tile import TileContext
from concourse.bass_utils import run_bass_kernel_spmd

# Problem shape (hardcoded per contract)
B, S, H, I = 16, 1024, 1408, 5632
EPS = 1e-6
NCORES = 8
P = 128
T_CORE = (B * S) // NCORES       # 2048 tokens per core
KH = H // P                      # 11 k-tiles of H
KI = I // P                      # 44 k-tiles of I
IC = 4                           # i-chunks in down phase
ISUB = KI // IC                  # 11 i-subtiles per chunk
TB = 1024                        # tokens per block
NB = T_CORE // TB                # 2 blocks
NCH = TB // 512                  # 512-col matmul chunks per psum tile

DT = mybir.dt.float16
DT_NP = np.float16


def build_bass():
    nc = bacc.Bacc(None, target_bir_lowering=False)

    # weights are host-repacked so each DMA is HBM-contiguous with long
    # per-partition runs: wup[ip] covers i-tiles {2ip, 2ip+1}
    # nt is packed [KH, 4, P, 512] so every (k, 512-token chunk) tile is a
    # single contiguous 128KB HBM read (the old [H, T] view produced 1KB
    # reads at 4KB stride, which measured ~2x slower during the startup
    # burst). wup is [KI/2, 2, P, KH, P] so half-tiles are contiguous.
    nt = nc.declare_dram_parameter("nt", [KH, 4, P, 512], DT, isOutput=False)
    warm = nc.declare_dram_parameter("warm", [P, 512], DT, isOutput=False)
    wup = nc.declare_dram_parameter("wup", [KI // 2, 2, P, KH, P], DT, isOutput=False)
    wdn = nc.declare_dram_parameter("wdn", [KH, P, IC, ISUB, P], DT, isOutput=False)
    bup = nc.declare_dram_parameter("bup", [I], mybir.dt.float32, isOutput=False)
    ot = nc.declare_dram_parameter("ot", [H, T_CORE], mybir.dt.float32, isOutput=True)

    wupT = wup.rearrange("i h p k q -> i p h k q")    # [KI/2, P, 2, KH, P] view
    bup2 = bup.rearrange("(i p) -> p i", p=P)         # [128, KI]

    def up_chain(nc, psu, actp, wupb, h2, i, ntk, act_tiles, bup_sb, col0):
        """One 512-col up chain + gelu writing act[:, col0:col0+512]."""
        ps = psu.tile([P, 512], mybir.dt.float32, tag="psu", name=f"ps{i}")
        for k in range(KH):
            nc.tensor.matmul(
                ps[:],
                wupb[:, h2, k],
                ntk[k][:],
                start=(k == 0),
                stop=(k == KH - 1),
            )
        if act_tiles[i] is None:
            act_tiles[i] = actp.tile([P, TB], DT, tag="act", name=f"act{i}")
        nc.scalar.activation(
            act_tiles[i][:, col0 : col0 + 512],
            ps[:],
            mybir.ActivationFunctionType.Gelu,
            bias=bup_sb[:, i : i + 1],
            scale=1.0,
        )

    with TileContext(nc) as tc:
        with (
            tc.tile_pool(name="const", bufs=2) as constp,
            tc.tile_pool(name="ntp", bufs=2 * KH) as ntp,
            tc.tile_pool(name="wupp", bufs=6) as wupp,
            tc.tile_pool(name="wdnp", bufs=2) as wdnp,
            tc.tile_pool(name="actp", bufs=KI + 2) as actp,
            tc.tile_pool(name="outp", bufs=4) as outp,
            tc.tile_pool(name="psu", bufs=3, space="PSUM") as psu,
            tc.tile_pool(name="psd", bufs=2, space="PSUM") as psd,
        ):
            # HAM warmup: a 128KB zeros tile leads the sync queue, then a
            # dead-end matmul chain keeps the PE busy from ~9.5us so the
            # clock gate reaches 8/8 (2.4 GHz) before the real chains
            # start, instead of ~20us into them.
            warm_sb = constp.tile([P, 512], DT, name="warm_sb")
            nc.sync.dma_start(out=warm_sb[:], in_=warm[:])
            bup_sb = constp.tile([P, KI], mybir.dt.float32)
            nc.gpsimd.dma_start(out=bup_sb[:], in_=bup2)
            wps = psu.tile([P, 512], mybir.dt.float32, tag="psu", name="wps")
            # 40 MMs ~= 8 cold (3.4us, fires the clock gate) + 32 warm
            # (6.9us): ends ~19.7us, by which the startup stream (at the
            # measured ~0.25GB/us early device rate) has delivered nt-A
            # plus enough wup tiles that the real chains never stall
            NWARM = 40
            for j in range(NWARM):
                nc.tensor.matmul(
                    wps[:],
                    warm_sb[:, 0:P],
                    warm_sb[:],
                    start=(j == 0),
                    stop=(j == NWARM - 1),
                )

            def down_phase(half, act_tiles):
                tok0 = half * TB
                for m in range(KH):
                    wdnb = wdnp.tile([P, IC, ISUB, P], DT, tag="wdn", name=f"wdn{m}")
                    nc.scalar.dma_start(out=wdnb[:], in_=wdn[m])
                    for c in range(NCH):
                        cs = slice(c * 512, (c + 1) * 512)
                        ps2 = psd.tile([P, 512], mybir.dt.float32, tag="psd", name=f"ps2_{m}_{c}")
                        for i in range(KI):
                            nc.tensor.matmul(
                                ps2[:],
                                wdnb[:, i // ISUB, i % ISUB],
                                act_tiles[i][:, cs],
                                start=(i == 0),
                                stop=(i == KI - 1),
                            )
                        # copy + DMA per 512-chunk: chunk 0 drains while
                        # chunk 1's matmul chain runs (shrinks the tail).
                        # The very last tile goes out in 256-col pieces on
                        # the (idle, HWDGE) sync queue: the gpsimd SWDGE
                        # descriptor generation costs ~3us per 512-col DMA,
                        # which would land squarely in the kernel tail.
                        last = half == 1 and m == KH - 1
                        npieces = 2 if last else 1
                        for pc in range(npieces):
                            w = 512 // npieces
                            lo = c * 512 + pc * w
                            osb = outp.tile([P, w], mybir.dt.float32, tag="osb", name=f"osb{m}_{c}_{pc}")
                            nc.vector.tensor_copy(
                                out=osb[:], in_=ps2[:, pc * w : (pc + 1) * w]
                            )
                            eng = nc.sync if last else nc.gpsimd
                            eng.dma_start(
                                out=ot[m * P : (m + 1) * P, tok0 + lo : tok0 + lo + w],
                                in_=osb[:],
                            )

            # ============== half 0 (tokens 0:1024) ==============
            # Up runs as two 512-col passes (A then B) re-streaming the
            # up weights: that costs DMA bytes (which are free, we are
            # PE-bound) but halves the nt prefix the first matmul chain
            # has to wait for, so the PE starts ~10us earlier.
            wup_first = wupp.tile([P, 2, KH, P], DT, tag="wup", name="wupf")
            nc.scalar.dma_start(out=wup_first[:, 0], in_=wup[0, 0])
            ntA = []
            for k in range(KH):
                t = ntp.tile([P, 512], DT, tag="ntb", name=f"ntA{k}")
                nc.sync.dma_start(out=t[:], in_=nt[k, 0])
                ntA.append(t)
            nc.sync.dma_start(out=wup_first[:, 1], in_=wup[0, 1])
            # wup[1..5] at half-tile granularity (each half a contiguous
            # 360KB read) so no early chain gates on more than 0.36MB
            wup_second = wupp.tile([P, 2, KH, P], DT, tag="wup", name="wups")
            nc.scalar.dma_start(out=wup_second[:, 0], in_=wup[1, 0])
            nc.sync.dma_start(out=wup_second[:, 1], in_=wup[1, 1])

            actsAB = [None] * KI
            ntB = []
            for ip in range(KI // 2):
                if ip == 0:
                    wupb = wup_first
                elif ip == 1:
                    wupb = wup_second
                elif ip < 6:
                    # half-tile DMAs split across the queues: every chain
                    # gates on <=0.36MB, so no single early wait can reach
                    # the 3.4us HAM re-throttle window
                    wupb = wupp.tile([P, 2, KH, P], DT, tag="wup", name=f"wupA{ip}")
                    nc.scalar.dma_start(out=wupb[:, 0], in_=wup[ip, 0])
                    nc.sync.dma_start(out=wupb[:, 1], in_=wup[ip, 1])
                else:
                    wupb = wupp.tile([P, 2, KH, P], DT, tag="wup", name=f"wupA{ip}")
                    nc.sync.dma_start(out=wupb[:], in_=wupT[ip])
                if ip == 6:
                    # nt-B rides the scalar queue mid-upA. Not earlier:
                    # its 1.4MB would contend with the wup stream exactly
                    # while the early pairs are still draining the
                    # startup deficit (measured: a 4us stall there
                    # re-throttles the PE clock gate to 1.2GHz).
                    for k in range(KH):
                        t = ntp.tile([P, 512], DT, tag="ntb", name=f"ntB{k}")
                        nc.scalar.dma_start(out=t[:], in_=nt[k, 1])
                        ntB.append(t)
                for h2 in range(2):
                    up_chain(nc, psu, actp, wupb, h2, 2 * ip + h2, ntA, actsAB, bup_sb, 0)

            for ip in range(KI // 2):
                wupb = wupp.tile([P, 2, KH, P], DT, tag="wup", name=f"wupB{ip}")
                nc.sync.dma_start(out=wupb[:], in_=wupT[ip])
                for h2 in range(2):
                    up_chain(nc, psu, actp, wupb, h2, 2 * ip + h2, ntB, actsAB, bup_sb, 512)

            down_phase(0, actsAB)

            # ============== half 1 (tokens 1024:2048) ==============
            ntC = [[], []]
            for c in range(2):
                for k in range(KH):
                    t = ntp.tile([P, 512], DT, tag="ntb", name=f"ntC{c}_{k}")
                    nc.sync.dma_start(out=t[:], in_=nt[k, 2 + c])
                    ntC[c].append(t)
            actsC = [None] * KI
            for ip in range(KI // 2):
                wupb = wupp.tile([P, 2, KH, P], DT, tag="wup", name=f"wupC{ip}")
                nc.sync.dma_start(out=wupb[:], in_=wupT[ip])
                for c in range(2):
                    for h2 in range(2):
                        up_chain(
                            nc, psu, actp, wupb, h2, 2 * ip + h2, ntC[c], actsC, bup_sb, c * 512
                        )
            down_phase(1, actsC)
    nc.compile()
    return nc


_CACHED = {}


def _get_nc():
    if "nc" not in _CACHED:
        _CACHED["nc"] = build_bass()
    return _CACHED["nc"]


def _prep_host(x, residual, ln_w, W_up, b_up, W_down):
    """Host-side prep: h, normed^T per core (fp16), repacked fp16 weights."""
    h = x + residual                                   # [B,S,H] f32
    hf = h.reshape(-1, H)                              # [16384, H]
    var = np.mean(np.square(hf), axis=-1)              # f32
    s = 1.0 / np.sqrt(var + EPS)                       # f32
    normed = (hf * s[:, None]).astype(DT_NP)           # fp16 (ln_w folded into W)

    Wup_p = (W_up * ln_w[:, None]).astype(DT_NP)       # [H, I] fp16
    # wup[ip, b, p, k, il] = Wup_p[k*128+p, (2*ip+b)*128+il]
    WUP = np.ascontiguousarray(
        Wup_p.reshape(KH, P, KI // 2, 2, P).transpose(2, 3, 1, 0, 4)
    )                                                  # [KI/2,2,P,KH,P] fp16
    # wdn[m, p, ic, isub, c] = W_down[(ic*ISUB+isub)*128+p, m*128+c]
    WDN = np.ascontiguousarray(
        W_down.astype(DT_NP).reshape(IC, ISUB, P, KH, P).transpose(3, 2, 0, 1, 4)
    )                                                  # [KH,P,IC,ISUB,P] fp16

    in_maps = []
    for c in range(NCORES):
        # [KH, 4, P, 512]: ntc[k, ch, p, t] = normed[c*T+ch*512+t, k*128+p]
        ntc = np.ascontiguousarray(
            normed[c * T_CORE : (c + 1) * T_CORE]
            .reshape(4, 512, KH, P)
            .transpose(2, 0, 3, 1)
        )
        in_maps.append(
            {
                "nt": ntc,
                "warm": np.zeros((P, 512), dtype=DT_NP),
                "wup": WUP,
                "wdn": WDN,
                "bup": b_up.astype(np.float32),
            }
        )
    return h, in_maps


_RESET_DONE = {}


def _maybe_reset_device():
    """Best-effort terminal NRT reset so a previously wedged device can't
    hang the run. No-op when the axon .so or symbol is unavailable."""
    if _RESET_DONE:
        return
    _RESET_DONE["done"] = True
    try:
        import ctypes
        import jax

        jax.devices()
        lib = ctypes.CDLL("/opt/axon/libaxon_pjrt.so")
        if hasattr(lib, "axon_reset"):
            lib.axon_reset.restype = ctypes.c_int64
            lib.axon_reset()
    except Exception:
        pass


def _run(in_maps, **kw):
    _maybe_reset_device()
    nc = _get_nc()
    return run_bass_kernel_spmd(nc, in_maps, core_ids=list(range(NCORES)), **kw)


def _assemble(results, b_down):
    outs = [r["ot"].T for r in results]                # each [T_CORE, H]
    out = np.concatenate(outs, axis=0).reshape(B, S, H)
    out = out + b_down.astype(np.float32)
    return out


def kernel(x, residual, ln_w, W_up, b_up, W_down, b_down):
    x = np.asarray(x, dtype=np.float32)
    residual = np.asarray(residual, dtype=np.float32)
    ln_w = np.asarray(ln_w, dtype=np.float32)
    W_up = np.asarray(W_up, dtype=np.float32)
    b_up = np.asarray(b_up, dtype=np.float32)
    W_down = np.asarray(W_down, dtype=np.float32)
    b_down = np.asarray(b_down, dtype=np.float32)

    h, in_maps = _prep_host(x, residual, ln_w, W_up, b_up, W_down)
    res = _run(in_maps)
    out = _assemble(res.results, b_down)
    return out, h


def kernel_traced(x, residual, ln_w, W_up, b_up, W_down, b_down, **kw):
    """Like kernel() but with NTFF tracing; returns ((out, h), results)."""
    h, in_maps = _prep_host(
        np.asarray(x, np.float32),
        np.asarray(residual, np.float32),
        np.asarray(ln_w, np.float32),
        np.asarray(W_up, np.float32),
        np.asarray(b_up, np.float32),
        np.asarray(W_down, np.float32),
    )
    res = _run(in_maps, trace=True, **kw)
    out = _assemble(res.results, np.asarray(b_down, np.float32))
    return (out, h), res


# revision 24
# speedup vs baseline: 1.0041x; 1.0041x over previous
"""Trainium2 Bass kernel for NeuronLlama4VisionMLP (fused residual-add +
RMSNorm + up-proj + GELU + down-proj).

Distribution: data-parallel over the 16384 tokens -> 2048 tokens per core,
full weights replicated per core, no collectives.

Host side (cheap elementwise / repack prep):
  - h = x + residual  (this is also the module's second output)
  - per-token rsqrt(mean(h^2)+eps) scale and ln_w are folded into the
    device inputs: normed = h * s, W_up' = ln_w[:,None] * W_up
  - normed is shipped transposed ([H, T] per core, fp16) so the device is
    a pure matmul pipeline; the device returns out^T and b_down is added
    on host.

Device side per core (T=2048 tokens, H=1408, I=5632), per TB-token block:
    up:   psum[i_tile, tok] = sum_k W_up'[k, i_tile].T @ normed_T[k, tok]
    gelu: act[i_tile] = Gelu(psum + b_up[i_tile])       (ACT engine)
    down: psum[m_tile, tok] = sum_i W_down[i, m_tile].T @ act[i]
    out^T[m_tile, tok] -> HBM

All matmul operands are fp16 (10-bit mantissa keeps fro rel-err ~4e-4):
fp16 weights enable fast weight load so LDWEIGHTS fully hides under the
512-column matmul (216 ns issue cadence vs 227 ns with f32r weights),
and halve HBM traffic vs f32.
"""
import sys

sys.path.insert(0, "/opt/trn_rl_repo")

import numpy as np
import ml_dtypes
import concourse.bass as bass
from concourse import bacc
import concourse.mybir as mybir
from concourse.tile import TileContext
from concourse.bass_utils import run_bass_kernel_spmd

# Problem shape (hardcoded per contract)
B, S, H, I = 16, 1024, 1408, 5632
EPS = 1e-6
NCORES = 8
P = 128
T_CORE = (B * S) // NCORES       # 2048 tokens per core
KH = H // P                      # 11 k-tiles of H
KI = I // P                      # 44 k-tiles of I
IC = 4                           # i-chunks in down phase
ISUB = KI // IC                  # 11 i-subtiles per chunk
TB = 1024                        # tokens per block
NB = T_CORE // TB                # 2 blocks
NCH = TB // 512                  # 512-col matmul chunks per psum tile

DT = mybir.dt.float16
DT_NP = np.float16


def build_bass():
    nc = bacc.Bacc(None, target_bir_lowering=False)

    # weights are host-repacked so each DMA is HBM-contiguous with long
    # per-partition runs: wup[ip] covers i-tiles {2ip, 2ip+1}
    # nt is packed [KH, 4, P, 512] so every (k, 512-token chunk) tile is a
    # single contiguous 128KB HBM read (the old [H, T] view produced 1KB
    # reads at 4KB stride, which measured ~2x slower during the startup
    # burst). wup is [KI/2, 2, P, KH, P] so half-tiles are contiguous.
    nt = nc.declare_dram_parameter("nt", [KH, 4, P, 512], DT, isOutput=False)
    warm = nc.declare_dram_parameter("warm", [P, 512], DT, isOutput=False)
    wup = nc.declare_dram_parameter("wup", [KI // 2, 2, P, KH, P], DT, isOutput=False)
    wdn = nc.declare_dram_parameter("wdn", [KH, P, IC, ISUB, P], DT, isOutput=False)
    bup = nc.declare_dram_parameter("bup", [I], mybir.dt.float32, isOutput=False)
    # ot is packed [KH, 4, P, 512] so each output tile is one contiguous
    # 256KB HBM write (the [H, T] layout fragmented into 1KB rows at 8KB
    # stride, stretching the final drain); host reassembles.
    ot = nc.declare_dram_parameter("ot", [KH, 4, P, 512], mybir.dt.float32, isOutput=True)

    wupT = wup.rearrange("i h p k q -> i p h k q")    # [KI/2, P, 2, KH, P] view
    bup2 = bup.rearrange("(i p) -> p i", p=P)         # [128, KI]

    def up_chain(nc, psu, actp, wupb, h2, i, ntk, act_tiles, bup_sb, col0):
        """One 512-col up chain + gelu writing act[:, col0:col0+512]."""
        ps = psu.tile([P, 512], mybir.dt.float32, tag="psu", name=f"ps{i}")
        for k in range(KH):
            nc.tensor.matmul(
                ps[:],
                wupb[:, h2, k],
                ntk[k][:],
                start=(k == 0),
                stop=(k == KH - 1),
            )
        if act_tiles[i] is None:
            act_tiles[i] = actp.tile([P, TB], DT, tag="act", name=f"act{i}")
        nc.scalar.activation(
            act_tiles[i][:, col0 : col0 + 512],
            ps[:],
            mybir.ActivationFunctionType.Gelu,
            bias=bup_sb[:, i : i + 1],
            scale=1.0,
        )

    with TileContext(nc) as tc:
        with (
            tc.tile_pool(name="const", bufs=2) as constp,
            tc.tile_pool(name="ntp", bufs=2 * KH) as ntp,
            tc.tile_pool(name="wupp", bufs=6) as wupp,
            tc.tile_pool(name="wdnp", bufs=2) as wdnp,
            tc.tile_pool(name="actp", bufs=KI + 2) as actp,
            tc.tile_pool(name="outp", bufs=4) as outp,
            tc.tile_pool(name="psu", bufs=3, space="PSUM") as psu,
            tc.tile_pool(name="psd", bufs=2, space="PSUM") as psd,
        ):
            # HAM warmup: a 128KB zeros tile leads the sync queue, then a
            # dead-end matmul chain keeps the PE busy from ~9.5us so the
            # clock gate reaches 8/8 (2.4 GHz) before the real chains
            # start, instead of ~20us into them.
            warm_sb = constp.tile([P, 512], DT, name="warm_sb")
            nc.sync.dma_start(out=warm_sb[:], in_=warm[:])
            bup_sb = constp.tile([P, KI], mybir.dt.float32)
            nc.gpsimd.dma_start(out=bup_sb[:], in_=bup2)
            wps = psu.tile([P, 512], mybir.dt.float32, tag="psu", name="wps")
            NWARM = 20
            for j in range(NWARM):
                nc.tensor.matmul(
                    wps[:],
                    warm_sb[:, 0:P],
                    warm_sb[:],
                    start=(j == 0),
                    stop=(j == NWARM - 1),
                )

            def down_phase(half, act_tiles):
                tok0 = half * TB
                for m in range(KH):
                    wdnb = wdnp.tile([P, IC, ISUB, P], DT, tag="wdn", name=f"wdn{m}")
                    nc.scalar.dma_start(out=wdnb[:], in_=wdn[m])
                    for c in range(NCH):
                        cs = slice(c * 512, (c + 1) * 512)
                        ps2 = psd.tile([P, 512], mybir.dt.float32, tag="psd", name=f"ps2_{m}_{c}")
                        for i in range(KI):
                            nc.tensor.matmul(
                                ps2[:],
                                wdnb[:, i // ISUB, i % ISUB],
                                act_tiles[i][:, cs],
                                start=(i == 0),
                                stop=(i == KI - 1),
                            )
                        # copy + DMA per 512-chunk: chunk 0 drains while
                        # chunk 1's matmul chain runs (shrinks the tail).
                        # The very last tile goes out on the (idle, HWDGE)
                        # sync queue: gpsimd SWDGE descriptor generation
                        # costs ~3us per DMA, which would land squarely in
                        # the kernel tail.
                        last = half == 1 and m == KH - 1
                        osb = outp.tile([P, 512], mybir.dt.float32, tag="osb", name=f"osb{m}_{c}")
                        nc.vector.tensor_copy(out=osb[:], in_=ps2[:])
                        eng = nc.sync if last else nc.gpsimd
                        eng.dma_start(out=ot[m, half * NCH + c], in_=osb[:])

            # ============== half 0 (tokens 0:1024) ==============
            # Up runs as two 512-col passes (A then B) re-streaming the
            # up weights: that costs DMA bytes (which are free, we are
            # PE-bound) but halves the nt prefix the first matmul chain
            # has to wait for, so the PE starts ~10us earlier.
            wup_first = wupp.tile([P, 2, KH, P], DT, tag="wup", name="wupf")
            nc.scalar.dma_start(out=wup_first[:, 0], in_=wup[0, 0])
            ntA = []
            for k in range(KH):
                t = ntp.tile([P, 512], DT, tag="ntb", name=f"ntA{k}")
                nc.sync.dma_start(out=t[:], in_=nt[k, 0])
                ntA.append(t)
            nc.sync.dma_start(out=wup_first[:, 1], in_=wup[0, 1])
            # wup[1..5] at half-tile granularity (each half a contiguous
            # 360KB read) so no early chain gates on more than 0.36MB
            wup_second = wupp.tile([P, 2, KH, P], DT, tag="wup", name="wups")
            nc.scalar.dma_start(out=wup_second[:, 0], in_=wup[1, 0])
            nc.sync.dma_start(out=wup_second[:, 1], in_=wup[1, 1])

            actsAB = [None] * KI
            ntB = []
            for ip in range(KI // 2):
                if ip == 0:
                    wupb = wup_first
                elif ip == 1:
                    wupb = wup_second
                elif ip < 6:
                    # half-tile DMAs split across the queues: every chain
                    # gates on <=0.36MB, so no single early wait can reach
                    # the 3.4us HAM re-throttle window
                    wupb = wupp.tile([P, 2, KH, P], DT, tag="wup", name=f"wupA{ip}")
                    nc.scalar.dma_start(out=wupb[:, 0], in_=wup[ip, 0])
                    nc.sync.dma_start(out=wupb[:, 1], in_=wup[ip, 1])
                else:
                    wupb = wupp.tile([P, 2, KH, P], DT, tag="wup", name=f"wupA{ip}")
                    nc.sync.dma_start(out=wupb[:], in_=wupT[ip])
                if ip == 6:
                    # nt-B rides the scalar queue mid-upA. Not earlier:
                    # its 1.4MB would contend with the wup stream exactly
                    # while the early pairs are still draining the
                    # startup deficit (measured: a 4us stall there
                    # re-throttles the PE clock gate to 1.2GHz).
                    for k in range(KH):
                        t = ntp.tile([P, 512], DT, tag="ntb", name=f"ntB{k}")
                        nc.scalar.dma_start(out=t[:], in_=nt[k, 1])
                        ntB.append(t)
                for h2 in range(2):
                    up_chain(nc, psu, actp, wupb, h2, 2 * ip + h2, ntA, actsAB, bup_sb, 0)

            for ip in range(KI // 2):
                wupb = wupp.tile([P, 2, KH, P], DT, tag="wup", name=f"wupB{ip}")
                nc.sync.dma_start(out=wupb[:], in_=wupT[ip])
                for h2 in range(2):
                    up_chain(nc, psu, actp, wupb, h2, 2 * ip + h2, ntB, actsAB, bup_sb, 512)

            down_phase(0, actsAB)

            # ============== half 1 (tokens 1024:2048) ==============
            ntC = [[], []]
            for c in range(2):
                for k in range(KH):
                    t = ntp.tile([P, 512], DT, tag="ntb", name=f"ntC{c}_{k}")
                    nc.sync.dma_start(out=t[:], in_=nt[k, 2 + c])
                    ntC[c].append(t)
            actsC = [None] * KI
            for ip in range(KI // 2):
                wupb = wupp.tile([P, 2, KH, P], DT, tag="wup", name=f"wupC{ip}")
                nc.sync.dma_start(out=wupb[:], in_=wupT[ip])
                for c in range(2):
                    for h2 in range(2):
                        up_chain(
                            nc, psu, actp, wupb, h2, 2 * ip + h2, ntC[c], actsC, bup_sb, c * 512
                        )
            down_phase(1, actsC)
    nc.compile()
    return nc


_CACHED = {}


def _get_nc():
    if "nc" not in _CACHED:
        _CACHED["nc"] = build_bass()
    return _CACHED["nc"]


def _prep_host(x, residual, ln_w, W_up, b_up, W_down):
    """Host-side prep: h, normed^T per core (fp16), repacked fp16 weights."""
    h = x + residual                                   # [B,S,H] f32
    hf = h.reshape(-1, H)                              # [16384, H]
    var = np.mean(np.square(hf), axis=-1)              # f32
    s = 1.0 / np.sqrt(var + EPS)                       # f32
    normed = (hf * s[:, None]).astype(DT_NP)           # fp16 (ln_w folded into W)

    Wup_p = (W_up * ln_w[:, None]).astype(DT_NP)       # [H, I] fp16
    # wup[ip, b, p, k, il] = Wup_p[k*128+p, (2*ip+b)*128+il]
    WUP = np.ascontiguousarray(
        Wup_p.reshape(KH, P, KI // 2, 2, P).transpose(2, 3, 1, 0, 4)
    )                                                  # [KI/2,2,P,KH,P] fp16
    # wdn[m, p, ic, isub, c] = W_down[(ic*ISUB+isub)*128+p, m*128+c]
    WDN = np.ascontiguousarray(
        W_down.astype(DT_NP).reshape(IC, ISUB, P, KH, P).transpose(3, 2, 0, 1, 4)
    )                                                  # [KH,P,IC,ISUB,P] fp16

    in_maps = []
    for c in range(NCORES):
        # [KH, 4, P, 512]: ntc[k, ch, p, t] = normed[c*T+ch*512+t, k*128+p]
        ntc = np.ascontiguousarray(
            normed[c * T_CORE : (c + 1) * T_CORE]
            .reshape(4, 512, KH, P)
            .transpose(2, 0, 3, 1)
        )
        in_maps.append(
            {
                "nt": ntc,
                "warm": np.zeros((P, 512), dtype=DT_NP),
                "wup": WUP,
                "wdn": WDN,
                "bup": b_up.astype(np.float32),
            }
        )
    return h, in_maps


_RESET_DONE = {}


def _maybe_reset_device():
    """Best-effort terminal NRT reset so a previously wedged device can't
    hang the run. No-op when the axon .so or symbol is unavailable."""
    if _RESET_DONE:
        return
    _RESET_DONE["done"] = True
    try:
        import ctypes
        import jax

        jax.devices()
        lib = ctypes.CDLL("/opt/axon/libaxon_pjrt.so")
        if hasattr(lib, "axon_reset"):
            lib.axon_reset.restype = ctypes.c_int64
            lib.axon_reset()
    except Exception:
        pass


def _run(in_maps, **kw):
    _maybe_reset_device()
    nc = _get_nc()
    return run_bass_kernel_spmd(nc, in_maps, core_ids=list(range(NCORES)), **kw)


def _assemble(results, b_down):
    outs = []
    for r in results:
        o = r["ot"]                                    # [KH, 4, P, 512]
        outs.append(o.transpose(1, 3, 0, 2).reshape(T_CORE, H))
    out = np.concatenate(outs, axis=0).reshape(B, S, H)
    out = out + b_down.astype(np.float32)
    return out


def kernel(x, residual, ln_w, W_up, b_up, W_down, b_down):
    x = np.asarray(x, dtype=np.float32)
    residual = np.asarray(residual, dtype=np.float32)
    ln_w = np.asarray(ln_w, dtype=np.float32)
    W_up = np.asarray(W_up, dtype=np.float32)
    b_up = np.asarray(b_up, dtype=np.float32)
    W_down = np.asarray(W_down, dtype=np.float32)
    b_down = np.asarray(b_down, dtype=np.float32)

    h, in_maps = _prep_host(x, residual, ln_w, W_up, b_up, W_down)
    res = _run(in_maps)
    out = _assemble(res.results, b_down)
    return out, h


def kernel_traced(x, residual, ln_w, W_up, b_up, W_down, b_down, **kw):
    """Like kernel() but with NTFF tracing; returns ((out, h), results)."""
    h, in_maps = _prep_host(
        np.asarray(x, np.float32),
        np.asarray(residual, np.float32),
        np.asarray(ln_w, np.float32),
        np.asarray(W_up, np.float32),
        np.asarray(b_up, np.float32),
        np.asarray(W_down, np.float32),
    )
    res = _run(in_maps, trace=True, **kw)
    out = _assemble(res.results, np.asarray(b_down, np.float32))
    return (out, h), res
